# revision 62
# baseline (speedup 1.0000x reference)
"""Self-contained Trainium2 Bass kernel: single-head attention.

Reference computation (per batch item b of 4, seq 4096, hidden 768, head 64):
    q = x@Wq+bq; k = x@Wk+bk; v = x@Wv+bv
    A = softmax(q k^T / 8); out = (A v) @ Wo + bo

Sharding over 8 NeuronCores: core c handles batch item c//2 and query rows
(c%2)*2048 .. +2048.  Each core computes Q^T/K^T/V^T for its own 2048 rows
from x^T (built on-chip via PE transposes), then the two cores of a pair
exchange K^T and V (natural layout) with pairwise AllGathers so each core
attends over the full 4096 keys.

Everything stays in "transposed" layout so matmul contractions line up:
  - proj:  PSUM[0:64]=Q^T, [64:128]=K^T via packed weights [Wq|Wk] (768x128)
  - S^T[k_tile, q] = matmul(lhsT=K^T slice, rhs=Q^T)        (scores transposed)
  - E^T = exp(S^T * 1/8)  on ScalarE, straight out of PSUM
  - ctx^T[65, q] += matmul(lhsT=[V|1] (128x65), rhs=E^T)    (row 64 = softmax
    denominator, for free)
  - normalize ctx^T by its own row 64, then
    out[q,768] = matmul(lhsT=ctx_n^T (65 rows, last=1.0), rhs=[Wo;bo])

All matmul operands are float32r (TF32-class PE mode, 4x the fp32 rate;
overall rel err ~1e-4); producers round on write, PSUM accumulation stays
fp32.  Queries run in two halves of 1024 columns; the first half's
normalize/output-projection/DMA is interleaved into the second half's
attention loop so only the final half's epilogue is exposed.

Communication/overlap structure:
  - K^T is exchanged in two pairwise AllGathers sized 3/4 + 1/4: the big
    first one covers key tiles {0..11, 16..27} and is in flight before the
    prologue finishes, so attention starts immediately; V follows, and the
    small K remainder only has to land 24 key tiles into the loop.
  - Collective-dependent readback DMAs ride the gpsimd/SWDGE ring (which
    already blocks on collective completion) so they never head-of-line
    block the SP HWDGE ring that streams x loads and output stores.
  - A deep E^T tile pool (14 bufs) plus deferred ctx matmuls lets ScalarE
    keep producing exp() tiles while the V exchange is still in flight.
  - A few throwaway matmuls at kernel start lift the PE out of its cold
    HAM clock state before the transposes land.

Scheduling details that matter (engines execute their streams in order):
  - S matmuls are emitted one iteration ahead of their exp so they land at
    the start of the previous exp's window on PE.
  - ctx matmuls are deferred 12 iterations behind the exps in the first
    query half (so no PE instruction that depends on the in-flight V
    exchange can block the S stream) and 1 iteration in the second.
  - The final epilogue rotates output-projection PSUM tiles across four
    then-idle slots and alternates copy engines so the output-DMA stream
    stays saturated.

Engine budget per core (cost model): ScalarE ~87us (64 exps of [128,1024]
dominate - the hard floor), PE ~85us, DVE ~29us, DMA ~52us; simulated
schedule ~128us.
"""

from contextlib import ExitStack

import numpy as np

import concourse.bacc as bacc
import concourse.mybir as mybir
import concourse.tile as tile
from concourse import bass_utils
from concourse.masks import make_identity

F32 = mybir.dt.float32
F32R = mybir.dt.float32r
AF = mybir.ActivationFunctionType
ADD = mybir.AluOpType.add

B, S, H, D = 4, 4096, 768, 64
SH = S // 2  # 2048 query rows per core
HQ = SH // 2  # 1024 query columns per half
HC = H // 128  # 6 hidden chunks
NKT = S // 128  # 32 key tiles (full sequence)
NST = SH // 128  # 16 seq tiles per core
NQB = SH // 512  # 4 blocks of 512 query columns
SCALE = 1.0 / float(np.sqrt(D))

_CACHE: dict = {}


def _kernel_body(tc, x_sh, wqk, bqk, wv, bv, woe, out, sim_mode=False):
    """sim_mode=True replaces the pairwise AllGathers with local DRAM copies
    of the same volume so the module can run under TimelineSim (which rejects
    collectives). Timing-only variant, never used for correctness."""
    nc = tc.nc
    with ExitStack() as ctx:
        consts = ctx.enter_context(tc.tile_pool(name="consts", bufs=1))
        ident = consts.tile([128, 128], F32)
        nc.vector.memset(ident, 0.0)
        make_identity(nc, ident, nomemset=True)
        # weights: DMA f32 staging, then one DVE cast-copy to f32r
        wqk_sb = consts.tile([128, HC, 128], F32R)
        wv_sb = consts.tile([128, HC, D], F32R)
        woe_sb = consts.tile([D + 1, H], F32R)
        # weight DMAs ride the ACT HWDGE ring so x loads start immediately on SP
        wstage = consts.tile([128, HC * 128 + HC * D + 8], F32, tag="wstage")
        nc.scalar.dma_start(
            out=wstage[:, 0 : HC * 128].rearrange("p (c m) -> p c m", c=HC),
            in_=wqk.rearrange("(c p) m -> p c m", p=128),
        )
        nc.scalar.dma_start(
            out=wstage[:, HC * 128 : HC * 128 + HC * D].rearrange(
                "p (c m) -> p c m", c=HC
            ),
            in_=wv.rearrange("(c p) m -> p c m", p=128),
        )
        nc.gpsimd.tensor_copy(
            out=wqk_sb,
            in_=wstage[:, 0 : HC * 128].rearrange("p (c m) -> p c m", c=HC),
        )
        nc.gpsimd.tensor_copy(
            out=wv_sb,
            in_=wstage[:, HC * 128 : HC * 128 + HC * D].rearrange(
                "p (c m) -> p c m", c=HC
            ),
        )
        wstage2 = consts.tile([D + 1, H], F32, tag="wstage2")
        nc.scalar.dma_start(out=wstage2, in_=woe)
        nc.gpsimd.tensor_copy(out=woe_sb, in_=wstage2)
        bqk_sb = consts.tile([128, 1], F32)
        nc.scalar.dma_start(out=bqk_sb, in_=bqk)
        bv_sb = consts.tile([D, 1], F32)
        nc.scalar.dma_start(out=bv_sb, in_=bv)
        onesF = consts.tile([128, D + 1], F32, tag="onesF")
        nc.vector.memset(onesF, 1.0)
        ones_sb = consts.tile([1, D + 1], F32R)
        nc.gpsimd.tensor_copy(out=ones_sb, in_=onesF[0:1, :])

        qkT = consts.tile([128, SH], F32R)  # rows 0:64 Q^T, 64:128 K^T(own)
        vT = consts.tile([D, SH], F32)  # V^T own
        vstage = consts.tile([128, NST, D], F32R)  # V own, natural layout
        kT = consts.tile([D, S], F32R)  # K^T, full sequence
        vext = consts.tile([128, NKT, D + 1], F32R)  # [V | 1] per key tile

        dram = ctx.enter_context(tc.tile_pool(name="ccdram", bufs=1, space="DRAM"))
        K1W = 3 * SH // 4  # 1536: own columns covered by the first K exchange
        K2W = SH - K1W  # 512
        ccK_in1 = dram.tile([D, K1W], F32R)
        ccK_out1 = dram.tile([2, D, K1W], F32R)
        ccK_in2 = dram.tile([D, K2W], F32R)
        ccK_out2 = dram.tile([2, D, K2W], F32R)
        ccV_in = dram.tile([128, NST * D], F32R)  # partition-major, contiguous
        ccV_out = dram.tile([2, 128, NST * D], F32R)
        rg = [[0, 1], [2, 3], [4, 5], [6, 7]]

        # PE warmup: dummy matmuls (results never read) to lift the PE out of
        # its cold clock state before the real work lands.
        dummy = consts.tile([128, 512], F32, tag="warm_dummy")
        nc.vector.memset(dummy, 0.0)
        with ExitStack() as warm:
            wpsum = warm.enter_context(
                tc.tile_pool(name="warm_psum", bufs=1, space="PSUM")
            )
            wp = wpsum.tile([128, 512], F32)
            for _ in range(4):
                nc.tensor.matmul(wp, ident, dummy, start=True, stop=True)

        # ---- phase 1: load x, transpose to x^T, project Q/K/V^T ----
        with ExitStack() as ph1:
            xstage = ph1.enter_context(tc.tile_pool(name="xstage", bufs=4))
            xt_pool = ph1.enter_context(tc.tile_pool(name="xt", bufs=1))
            tp_psum = ph1.enter_context(
                tc.tile_pool(name="tp_psum", bufs=4, space="PSUM")
            )
            pj_psum = ph1.enter_context(
                tc.tile_pool(name="pj_psum", bufs=2, space="PSUM")
            )
            xT = xt_pool.tile([128, HC, SH], F32R)  # x^T (hidden on partitions)
            xas = []
            for g in range(NQB):
                xa = xstage.tile([128, 4, H], F32, tag="xa")
                xas.append(xa)
                if g == 0:
                    for hhalf in range(2):
                        nc.sync.dma_start(
                            out=xa[:, hhalf * 2 : hhalf * 2 + 2, :],
                            in_=x_sh[hhalf * 256 : (hhalf + 1) * 256, :].rearrange(
                                "(t p) h -> p t h", p=128
                            ),
                        )
                else:
                    nc.sync.dma_start(
                        out=xa,
                        in_=x_sh[g * 512 : (g + 1) * 512, :].rearrange(
                            "(t p) h -> p t h", p=128
                        ),
                    )

            def v_group(g):
                pv = pj_psum.tile([D, 512], F32, tag="pv")
                for j in range(HC):
                    nc.tensor.matmul(
                        pv,
                        wv_sb[:, j, :],
                        xT[:, j, g * 512 : (g + 1) * 512],
                        start=(j == 0),
                        stop=(j == HC - 1),
                    )
                nc.vector.tensor_scalar(
                    out=vT[:, g * 512 : (g + 1) * 512],
                    in0=pv,
                    scalar1=bv_sb,
                    scalar2=None,
                    op0=ADD,
                )
                ptv = tp_psum.tile([128, 512], F32, tag="tp")
                for t in range(4):
                    st = g * 4 + t
                    nc.tensor.transpose(
                        ptv[:, t * 128 : t * 128 + D],
                        vT[:, st * 128 : (st + 1) * 128],
                        ident[:D, :D],
                    )
                for t in range(4):
                    nc.scalar.copy(
                        out=vstage[:, g * 4 + t, :],
                        in_=ptv[:, t * 128 : t * 128 + D],
                    )

            for g in range(NQB):
                xa = xas[g]
                for j in range(HC):
                    pt = tp_psum.tile([128, 512], F32, tag="tp")
                    for t in range(4):
                        nc.tensor.transpose(
                            pt[:, t * 128 : (t + 1) * 128],
                            xa[:, t, j * 128 : (j + 1) * 128],
                            ident,
                        )
                    dst = xT[:, j, g * 512 : (g + 1) * 512]
                    if j % 3 == 0:
                        nc.vector.tensor_copy(out=dst, in_=pt)
                    else:
                        nc.scalar.copy(out=dst, in_=pt)
                # QK projection for this block of 512 query columns
                pq = pj_psum.tile([128, 512], F32, tag="pqk")
                for j in range(HC):
                    nc.tensor.matmul(
                        pq,
                        wqk_sb[:, j, :],
                        xT[:, j, g * 512 : (g + 1) * 512],
                        start=(j == 0),
                        stop=(j == HC - 1),
                    )
                nc.vector.tensor_scalar(
                    out=qkT[:, g * 512 : (g + 1) * 512],
                    in0=pq,
                    scalar1=bqk_sb,
                    scalar2=None,
                    op0=ADD,
                )
                if g >= 2:
                    # groups 0-1's V work runs after the K1 data is complete
                    v_group(g - 2)

            # first 3/4 of K^T (groups 0-2) exchanges as soon as it exists,
            # emitted after the x loads so its data-wait never head-of-line
            # blocks the SP DMA ring
            nc.sync.dma_start(out=ccK_in1[:, :], in_=qkT[64:128, 0:K1W])
            if sim_mode:
                nc.sync.dma_start(out=ccK_out1[0], in_=ccK_in1)
                nc.sync.dma_start(out=ccK_out1[1], in_=ccK_in1)
            else:
                nc.gpsimd.collective_compute(
                    "AllGather",
                    mybir.AluOpType.bypass,
                    replica_groups=rg,
                    ins=[ccK_in1.opt()],
                    outs=[ccK_out1.opt()],
                )
            nc.gpsimd.dma_start(out=kT[:, 0:K1W], in_=ccK_out1[0])
            nc.gpsimd.dma_start(out=kT[:, SH : SH + K1W], in_=ccK_out1[1])

            # remaining V groups, then exchange
            v_group(2)
            v_group(3)
            nc.sync.dma_start(
                out=ccV_in, in_=vstage.rearrange("p st d -> p (st d)")
            )
            if sim_mode:
                nc.sync.dma_start(out=ccV_out[0], in_=ccV_in)
                nc.sync.dma_start(out=ccV_out[1], in_=ccV_in)  # stub: own data twice
            else:
                nc.gpsimd.collective_compute(
                    "AllGather",
                    mybir.AluOpType.bypass,
                    replica_groups=rg,
                    ins=[ccV_in.opt()],
                    outs=[ccV_out.opt()],
                )

            # V readback rides the SP HWDGE ring: from here until the
            # epilogue stores, SP has nothing else queued, so its wait on the
            # collective blocks nothing.  The DMA stays fully contiguous; the
            # (then-idle) DVE unpacks into the 65-strided [V|1] layout.
            vtmp = consts.tile([128, 2, NST * D], F32R, tag="vtmp")
            for r in range(2):
                nc.sync.dma_start(out=vtmp[:, r, :], in_=ccV_out[r])
                nc.vector.tensor_copy(
                    out=vext[:, r * NST : (r + 1) * NST, 0:D],
                    in_=vtmp[:, r, :].rearrange("p (st d) -> p st d", d=D),
                )

            # remaining quarter of K^T (needed 24 key tiles into the loop)
            nc.sync.dma_start(out=ccK_in2[:, :], in_=qkT[64:128, K1W:SH])
            if sim_mode:
                nc.sync.dma_start(out=ccK_out2[0], in_=ccK_in2)
                nc.sync.dma_start(out=ccK_out2[1], in_=ccK_in2)
            else:
                nc.gpsimd.collective_compute(
                    "AllGather",
                    mybir.AluOpType.bypass,
                    replica_groups=rg,
                    ins=[ccK_in2.opt()],
                    outs=[ccK_out2.opt()],
                )
            nc.sync.dma_start(out=kT[:, K1W:SH], in_=ccK_out2[0])
            nc.sync.dma_start(out=kT[:, SH + K1W : S], in_=ccK_out2[1])

        # ---- read back gathered V, build [V|1] ----
        nc.vector.tensor_copy(
            out=vext[:, :, D], in_=onesF[:, 0:NKT]
        )

        # ---- attention in two query halves + pipelined epilogues ----
        c_pool = ctx.enter_context(tc.tile_pool(name="c_psum", bufs=1, space="PSUM"))
        s_pool = ctx.enter_context(tc.tile_pool(name="s_psum", bufs=2, space="PSUM"))
        ep_psum = ctx.enter_context(tc.tile_pool(name="ep_psum", bufs=1, space="PSUM"))
        e_pool = ctx.enter_context(tc.tile_pool(name="e_sb", bufs=14))
        ep_sb = ctx.enter_context(tc.tile_pool(name="ep_sb", bufs=2))
        osb_pool = ctx.enter_context(tc.tile_pool(name="o_sb", bufs=4))

        def epilogue_pieces(h, ctxh, final=False):
            """Closures: normalize ctx^T then output-projection tiles.  The
            hidden epilogue (h=0) runs as one chunk interleaved into the next
            attention half; the final epilogue runs as two 512-column chunks
            so normalize/matmul/copy/DMA pipeline against each other, with
            PSUM slots rotating across the then-idle s_pool/c_pool."""
            state = {}

            def normalize(c0, cn):
                if "recip" not in state:
                    recip_t = ep_sb.tile([1, HQ], F32R, tag="recip", name="recip_t")
                    bcast_t = ep_sb.tile([D + 1, HQ], F32, tag="bcast", name="bcast_t")
                    ctxn_t = ep_sb.tile([D + 1, HQ], F32R, tag="ctxn", name="ctxn_t")
                    state["recip"], state["bcast"] = recip_t, bcast_t
                    state["ctxn"] = ctxn_t
                recip, bcast = state["recip"], state["bcast"]
                ctxn = state["ctxn"]
                with nc.allow_low_precision(reason="f32r softmax denom"):
                    nc.vector.reciprocal(
                        out=recip[:, c0 * 512 : (c0 + cn) * 512],
                        in_=ctxh[D : D + 1, c0 * 512 : (c0 + cn) * 512],
                    )
                bcp = ep_psum.tile([D + 1, HQ], F32, tag="ep")
                for c in range(c0, c0 + cn):
                    nc.tensor.matmul(
                        bcp[:, c * 512 : (c + 1) * 512],
                        ones_sb,
                        recip[:, c * 512 : (c + 1) * 512],
                        start=True,
                        stop=True,
                    )
                nc.scalar.copy(
                    out=bcast[:, c0 * 512 : (c0 + cn) * 512],
                    in_=bcp[:, c0 * 512 : (c0 + cn) * 512],
                )
                nc.vector.tensor_mul(
                    ctxn[:, c0 * 512 : (c0 + cn) * 512],
                    ctxh[:, c0 * 512 : (c0 + cn) * 512],
                    bcast[:, c0 * 512 : (c0 + cn) * 512],
                )

            def po_tile(qt):
                ctxn = state["ctxn"]
                if final and qt % 4 in (1, 3):
                    po = s_pool.tile([128, H], F32, tag="s")
                elif final and qt % 4 == 2:
                    po = c_pool.tile([128, H], F32, tag="ctx")
                else:
                    po = ep_psum.tile([128, H], F32, tag="ep")
                lhsT = ctxn[:, qt * 128 : (qt + 1) * 128]
                nc.tensor.matmul(
                    po[:, 0:512], lhsT, woe_sb[:, 0:512], start=True, stop=True
                )
                nc.tensor.matmul(
                    po[:, 512:H], lhsT, woe_sb[:, 512:H], start=True, stop=True
                )
                ob = osb_pool.tile([128, H], F32, tag="ob")
                if final and qt % 2 == 1:
                    nc.scalar.copy(out=ob, in_=po)
                else:
                    nc.vector.tensor_copy(out=ob, in_=po)
                row0 = h * HQ + qt * 128
                nc.sync.dma_start(out=out[row0 : row0 + 128, :], in_=ob)

            if final:
                pieces = [lambda: normalize(0, 1)]
                pieces += [(lambda qt=qt: po_tile(qt)) for qt in range(4)]
                pieces.insert(2, lambda: normalize(1, 1))
                pieces += [(lambda qt=qt: po_tile(qt)) for qt in range(4, 8)]
                return pieces
            return [lambda: normalize(0, 2)] + [
                (lambda qt=qt: po_tile(qt)) for qt in range(HQ // 128)
            ]

        def ctx_mms(ctxh, es, h, kt, ki):
            e = es.pop(kt)
            for hf in range(2):
                nc.tensor.matmul(
                    ctxh[:, hf * 512 : (hf + 1) * 512],
                    vext[:, kt, :],
                    e[:, hf * 512 : (hf + 1) * 512],
                    start=(ki == 0),
                    stop=(ki == NKT - 1),
                )

        KT_ORDER = (
            list(range(0, 12)) + list(range(16, 28))
            + list(range(12, 16)) + list(range(28, 32))
        )
        def s_mms(q0h, kt):
            # scores for one key tile: S^T[kt, q-half] (2 chunks of 512)
            sp = s_pool.tile([128, 1024], F32, tag="s")
            for hf in range(2):
                q0 = q0h + hf * 512
                nc.tensor.matmul(
                    sp[:, hf * 512 : (hf + 1) * 512],
                    kT[:, kt * 128 : (kt + 1) * 128],
                    qkT[0:D, q0 : q0 + 512],
                    start=True,
                    stop=True,
                )
            return sp

        pending = []
        sps = {}
        for h in range(2):
            q0h = h * HQ
            ctxh = c_pool.tile([D + 1, HQ], F32, tag="ctx")
            es = {}
            prev_kt = None
            # ctx matmuls are emitted DEPTH iterations behind the exps: PE
            # executes in order, so an early ctx waiting on the (still in
            # flight) V exchange would block every S matmul queued behind it
            # and starve ScalarE.  h=0 rides out the V latency with a deep
            # deferral; h=1 has V resident and keeps the pipeline tight.
            depth = 12 if h == 0 else 1
            for ki, kt in enumerate(KT_ORDER):
                # S matmuls are emitted one iteration ahead so they land at
                # the START of the previous exp's window on the PE stream
                if kt not in sps:
                    sps[kt] = s_mms(q0h, kt)
                sp = sps.pop(kt)
                e = e_pool.tile([128, 1024], F32R, tag="e")
                nc.scalar.activation(out=e, in_=sp, func=AF.Exp, scale=SCALE)
                es[kt] = e
                if ki + 1 < NKT:
                    sps[KT_ORDER[ki + 1]] = s_mms(q0h, KT_ORDER[ki + 1])
                elif h == 0:
                    sps[KT_ORDER[0]] = s_mms(HQ, KT_ORDER[0])
                if ki >= depth:
                    ctx_mms(ctxh, es, h, KT_ORDER[ki - depth], ki - depth)
                # interleave the previous half's epilogue into this half
                # (one piece every other iteration: a piece's ~0.5us of PE
                # work exceeds the per-iteration PE slack vs the exp stream)
                if pending and 2 <= ki and ki % 2 == 0:
                    pending.pop(0)()
            for ki in range(NKT - depth, NKT):
                ctx_mms(ctxh, es, h, KT_ORDER[ki], ki)
            while pending:
                pending.pop(0)()
            pending = epilogue_pieces(h, ctxh, final=(h == 1))
        while pending:
            pending.pop(0)()


def _build(sim_mode=False):
    nc = bacc.Bacc(
        "TRN2",
        target_bir_lowering=False,
        debug=False,
        num_devices=1 if sim_mode else 8,
    )
    x_sh = nc.dram_tensor("x_sh", [SH, H], F32, kind="ExternalInput").ap()
    wqk = nc.dram_tensor("wqk", [H, 128], F32, kind="ExternalInput").ap()
    bqk = nc.dram_tensor("bqk", [128, 1], F32, kind="ExternalInput").ap()
    wv = nc.dram_tensor("wv", [H, D], F32, kind="ExternalInput").ap()
    bv = nc.dram_tensor("bv", [D, 1], F32, kind="ExternalInput").ap()
    woe = nc.dram_tensor("woe", [D + 1, H], F32, kind="ExternalInput").ap()
    out = nc.dram_tensor("out", [SH, H], F32, kind="ExternalOutput").ap()

    with tile.TileContext(nc) as tc:
        _kernel_body(tc, x_sh, wqk, bqk, wv, bv, woe, out, sim_mode=sim_mode)
    nc.compile()
    return nc


def get_nc():
    if "nc" not in _CACHE:
        _CACHE["nc"] = _build()
    return _CACHE["nc"]


def make_in_maps(x, Wq, bq, Wk, bk, Wv, bv, Wo, bo):
    f = lambda a: np.ascontiguousarray(np.asarray(a, dtype=np.float32))
    x = f(x)
    wqk = np.concatenate([f(Wq), f(Wk)], axis=1)  # [768, 128]
    bqk = np.concatenate([f(bq), f(bk)])[:, None]  # [128, 1]
    woe = np.concatenate([f(Wo), f(bo)[None, :]], axis=0)  # [65, 768]
    wv_ = f(Wv)
    bv_ = f(bv)[:, None]
    in_maps = []
    for c in range(8):
        b, h = c // 2, c % 2
        in_maps.append(
            {
                "x_sh": np.ascontiguousarray(x[b, h * SH : (h + 1) * SH, :]),
                "wqk": wqk,
                "bqk": bqk,
                "wv": wv_,
                "bv": bv_,
                "woe": woe,
            }
        )
    return in_maps


def kernel(x, Wq, bq, Wk, bk, Wv, bv, Wo, bo, trace=False):
    nc = get_nc()
    in_maps = make_in_maps(x, Wq, bq, Wk, bk, Wv, bv, Wo, bo)
    res = bass_utils.run_bass_kernel_spmd(
        nc, in_maps, core_ids=list(range(8)), trace=trace
    )
    out = np.empty((B, S, H), dtype=np.float32)
    for c in range(8):
        b, h = c // 2, c % 2
        out[b, h * SH : (h + 1) * SH, :] = res.results[c]["out"]
    if trace:
        return out, res
    return out
